# revision 15
# baseline (speedup 1.0000x reference)
"""Trainium2 Bass kernel for nn_DIDAModule (dense_cnn).

Math: the per-sample "dynamic" depthwise kernels are affine in the channel
gate g:  kern1 = g*A1 + B1  with  A1 = wk*wck, B1 = bk*wck + bck  (5x5) and
A2 = wk2*wck2, B2 = bk2*wck2 + bck2 (3x3, dilation 2).  A/B are shared across
channels, and g is constant per (sample, channel), so the gate commutes with
the spatial conv:

    o1 = g * conv_A1(f) + conv_B1(f)      o2 = g * conv_A2(f) + conv_B2(f)
    y  = [W_fuse @ o1 + b_fuse ; W_fuse @ o2 + b_fuse]

Only ONE feature map (f) is ever convolved.  Pipeline per sample:
  1. conv1 (1x1): stationary W_conv^T chunks, moving x (f32r, full rate),
     psum -> relu -> channel-major f (bf16).  The gate g comes from DVE
     reductions of the same psums (pre-relu spatial mean).
  2. PE-transpose f into spatial-major fT blocks (128-pixel flat blocks,
     25 blocks of the 56x56 map padded to 3200), staged via PSUM.
  3. Banded-conv sweep: stationary fT blocks, moving host-built band
     matrices T[phase,pos] for the 4 static kernels (A1,A2,B1,B2) -> psum
     [c, 4*128] per output block, accumulated over 3 input-block positions
     (fp8 DoubleRow pairs two positions per matmul when SWEEP_FP8).
  4. Combine on DVE: o_br = psA*g + psB (scalar_tensor_tensor), bf16.
  5. Fuse: stationary W_fuse^T chunks, moving o (channel-major), bias via
     activation, y out in bf16.

No DMA transposes anywhere (the xbar path needs fences + delay chains on
this HW); the only DRAM traffic is x in and y out.

Sharding: data-parallel over batch N across the 8 cores (4 samples each),
weights replicated.
"""

import os
import numpy as np

# ---------------------------------------------------------------- dims
N, C, H, W = 32, 512, 56, 56
CM, K1, K2, P2 = 128, 5, 3, 256
HW = H * W            # 3136
SP = 3200             # padded spatial: 25 blocks of 128
NB = 25
PH = 7                # phase classes (128 mod 56 = 16, period 7)
NCORES = 8
NPC = N // NCORES     # samples per core
SCH = 448             # conv free chunk: 3136 = 7*448
TSCALE = 4096.0       # fp8 band-matrix scale (folded out of W_fuse)

SWEEP_FP8 = bool(int(os.environ.get("CCK_FP8", "0")))  # fp8 f costs 1.9e-2 rel err
# x is cast to bf16 on the host: halves x DMA and keeps conv1 all-bf16
# (mixed f32r x bf16 matmuls are rejected by the compiler, NCC_IBIR034).
X_F32 = bool(int(os.environ.get("CCK_XF32", "0")))

_CACHE = {}


# ---------------------------------------------------------------- host prep
def _build_T(K2d, dil):
    """Banded conv matrices T[phase, pos, k_in, m_out] for flat 128-blocks."""
    kh = K2d.shape[0]
    r = (kh - 1) // 2 * dil
    T = np.zeros((PH, 3, 128, 128), np.float32)
    for p in range(PH):
        bref = 7 + p              # interior reference block of this phase
        for pos, d in enumerate((-1, 0, 1)):
            for m in range(128):
                s_out = bref * 128 + m
                ro, wo = divmod(s_out, W)
                for k in range(128):
                    s_in = (bref + d) * 128 + k
                    ri, wi = divmod(s_in, W)
                    di, dj = ri - ro, wi - wo
                    if (abs(di) <= r and abs(dj) <= r
                            and di % dil == 0 and dj % dil == 0):
                        T[p, pos, k, m] = K2d[di // dil + (kh - 1) // 2,
                                              dj // dil + (kh - 1) // 2]
    return T


def _host_consts(inp):
    import ml_dtypes
    bf16 = ml_dtypes.bfloat16
    f8 = ml_dtypes.float8_e4m3
    W_conv = np.asarray(inp["W_conv"], np.float32)     # [CM, C]
    W_fuse = np.asarray(inp["W_fuse"], np.float32)     # [P2, CM]
    A1 = (np.asarray(inp["wk"]) * float(inp["wck"])).reshape(K1, K1)
    B1 = (np.asarray(inp["bk"]) * float(inp["wck"]) + float(inp["bck"])).reshape(K1, K1)
    A2 = (np.asarray(inp["wk2"]) * float(inp["wck2"])).reshape(K2, K2)
    B2 = (np.asarray(inp["bk2"]) * float(inp["wck2"]) + float(inp["bck2"])).reshape(K2, K2)
    # kid order (A1, A2, B1, B2) so the combine can slice A=[0:2], B=[2:4]
    T4 = np.stack([_build_T(A1.astype(np.float32), 1),
                   _build_T(A2.astype(np.float32), 2),
                   _build_T(B1.astype(np.float32), 1),
                   _build_T(B2.astype(np.float32), 2)])   # [kid, ph, pos, k, m]
    # conv1 lhsT chunks: [c_local(128part), kc(4), cm(128)]
    wconvT_h = np.ascontiguousarray(
        W_conv.T.reshape(4, 128, CM).transpose(1, 0, 2))
    # fuse lhsT chunks: [c(128part), chunk(2), o_local(128)]
    wfuseT_h = np.ascontiguousarray(W_fuse.T.reshape(CM, 2, 128))
    d = {
        "wconvT": wconvT_h.astype(np.float32 if X_F32 else bf16),
        "bconv": np.asarray(inp["b_conv"], np.float32).reshape(CM, 1),
        "bfuseT": np.ascontiguousarray(
            np.asarray(inp["b_fuse"], np.float32).reshape(2, 128).T),  # [128, 2]
        "ident": np.eye(128, dtype=bf16),
    }
    if SWEEP_FP8:
        # DoubleRow pairs (pos0, pos1); pos2 rides a plain fp8 matmul.
        Tdr = np.ascontiguousarray(
            (T4[:, :, 0:2] * TSCALE).transpose(3, 1, 2, 0, 4)).astype(f8)
        Tsg = np.ascontiguousarray(
            (T4[:, :, 2] * TSCALE).transpose(2, 1, 0, 3)).astype(f8)
        d["Tdr"] = Tdr                       # [k, ph, t, kid, m]
        d["Tsg"] = Tsg                       # [k, ph, kid, m]
        d["wfuseT"] = (wfuseT_h / TSCALE).astype(bf16)
    else:
        d["Tbf"] = np.ascontiguousarray(
            T4.transpose(3, 1, 2, 0, 4)).astype(bf16)   # [k, ph, pos, kid, m]
        d["wfuseT"] = wfuseT_h.astype(bf16)
    return d


# ---------------------------------------------------------------- bass module
def _build_module():
    from contextlib import ExitStack
    import concourse.bass as bass  # noqa: F401
    import concourse.mybir as mybir
    import concourse.tile as tile
    from concourse import bacc

    dt = mybir.dt
    AX = mybir.AxisListType
    AF = mybir.ActivationFunctionType
    ALU = mybir.AluOpType
    DR = mybir.MatmulPerfMode.DoubleRow

    nc = bacc.Bacc("TRN2", target_bir_lowering=False, debug=False)

    reps = int(os.environ.get("CCK_REPS", "1"))
    x_dt = dt.float32r if X_F32 else dt.bfloat16
    w_dt = x_dt
    f8_dt = dt.float8e4

    x_d = nc.dram_tensor("x", [NPC, C, HW], x_dt, kind="ExternalInput").ap()
    wconvT_d = nc.dram_tensor("wconvT", [128, 4, CM], w_dt, kind="ExternalInput").ap()
    bconv_d = nc.dram_tensor("bconv", [CM, 1], dt.float32, kind="ExternalInput").ap()
    bfuseT_d = nc.dram_tensor("bfuseT", [128, 2], dt.float32, kind="ExternalInput").ap()
    ident_d = nc.dram_tensor("ident", [128, 128], dt.bfloat16, kind="ExternalInput").ap()
    wfuseT_d = nc.dram_tensor("wfuseT", [CM, 2, 128], dt.bfloat16, kind="ExternalInput").ap()
    if SWEEP_FP8:
        Tdr_d = nc.dram_tensor("Tdr", [128, PH, 2, 4, 128], f8_dt, kind="ExternalInput").ap()
        Tsg_d = nc.dram_tensor("Tsg", [128, PH, 4, 128], f8_dt, kind="ExternalInput").ap()
    else:
        Tbf_d = nc.dram_tensor("Tbf", [128, PH, 3, 4, 128], dt.bfloat16, kind="ExternalInput").ap()
    y_d = nc.dram_tensor("y", [NPC, 2 * P2, HW], dt.bfloat16, kind="ExternalOutput").ap()

    with tile.TileContext(nc) as tc, ExitStack() as ctx:
        consts = ctx.enter_context(tc.tile_pool(name="consts", bufs=1))
        xpool = ctx.enter_context(tc.tile_pool(name="xp", bufs=2))
        fpool = ctx.enter_context(tc.tile_pool(name="fp", bufs=2))
        opool = ctx.enter_context(tc.tile_pool(name="op", bufs=2))
        ypool = ctx.enter_context(tc.tile_pool(name="yp", bufs=3))
        small = ctx.enter_context(tc.tile_pool(name="sm", bufs=2))
        ps_c1 = ctx.enter_context(tc.tile_pool(name="psc1", bufs=2, space="PSUM"))
        ps_sq = ctx.enter_context(tc.tile_pool(name="pssq", bufs=2, space="PSUM"))
        ps_fu = ctx.enter_context(tc.tile_pool(name="psfu", bufs=2, space="PSUM"))

        # ---- constants to SBUF
        wconvT = consts.tile([128, 4, CM], w_dt)
        nc.sync.dma_start(out=wconvT, in_=wconvT_d)
        bconv = consts.tile([CM, 1], dt.float32)
        nc.sync.dma_start(out=bconv, in_=bconv_d)
        bfuseT = consts.tile([128, 2], dt.float32)
        nc.sync.dma_start(out=bfuseT, in_=bfuseT_d)
        ident = consts.tile([128, 128], dt.bfloat16)
        nc.sync.dma_start(out=ident, in_=ident_d)
        wfuseT = consts.tile([CM, 2, 128], dt.bfloat16)
        nc.sync.dma_start(out=wfuseT, in_=wfuseT_d)
        if SWEEP_FP8:
            Tdr = consts.tile([128, PH, 2, 4, 128], f8_dt)
            nc.sync.dma_start(out=Tdr, in_=Tdr_d)
            Tsg = consts.tile([128, PH, 4, 128], f8_dt)
            nc.sync.dma_start(out=Tsg, in_=Tsg_d)
        else:
            Tbf = consts.tile([128, PH, 3, 4, 128], dt.bfloat16)
            nc.sync.dma_start(out=Tbf, in_=Tbf_d)

        fT_dt = f8_dt if SWEEP_FP8 else dt.bfloat16

        for rep in range(reps):
          for n in range(NPC):
            # ---- x in
            xt = xpool.tile([128, 4, HW], x_dt, tag="x")
            for kc in range(4):
                nc.sync.dma_start(out=xt[:, kc, :], in_=x_d[n, kc * 128:(kc + 1) * 128, :])

            # ---- conv1 (channel-major f) + gate pieces
            f_cm = fpool.tile([128, SP], dt.bfloat16, tag="fcm")
            nc.gpsimd.memset(f_cm[:, HW:SP], 0.0)
            gpart = small.tile([128, 8], dt.float32, tag="gp")
            for sch in range(7):
                ps = ps_c1.tile([128, SCH], dt.float32, tag="c1")
                for kc in range(4):
                    nc.tensor.matmul(ps, wconvT[:, kc, :],
                                     xt[:, kc, sch * SCH:(sch + 1) * SCH],
                                     start=(kc == 0), stop=(kc == 3))
                nc.vector.reduce_sum(gpart[:, sch:sch + 1], ps, axis=AX.X)
                nc.scalar.activation(f_cm[:, sch * SCH:(sch + 1) * SCH], ps,
                                     AF.Relu, bias=bconv[:, 0:1], scale=1.0)
            gsum = small.tile([128, 1], dt.float32, tag="gs")
            nc.vector.reduce_sum(gsum, gpart[:, 0:7], axis=AX.X)
            g = small.tile([128, 1], dt.float32, tag="g")
            nc.scalar.activation(g, gsum, AF.Relu, bias=bconv[:, 0:1],
                                 scale=1.0 / HW)

            # ---- PE transpose into spatial-major fT (pad blocks 0/26 zero)
            fT = fpool.tile([128, NB + 2, 128], fT_dt, tag="fT")
            nc.gpsimd.memset(fT[:, 0, :], 0.0)
            nc.gpsimd.memset(fT[:, NB + 1, :], 0.0)
            for grp in range(4):
                w = 8 if grp < 3 else 1
                pst = ps_sq.tile([128, 2, 4, 128], dt.bfloat16, tag="sq")
                for b in range(w):
                    bo = 8 * grp + b
                    nc.tensor.matmul(pst[:, b // 4, b % 4, :],
                                     f_cm[:, bo * 128:(bo + 1) * 128],
                                     ident, is_transpose=True, skip_group_check=True)
                dst = fT[:, 1 + 8 * grp:1 + 8 * grp + w, :]
                src = pst.rearrange("p a b m -> p (a b) m")[:, 0:w, :]
                if grp % 2 == 0:
                    nc.scalar.activation(dst, src, AF.Copy)
                else:
                    nc.vector.tensor_copy(dst, src)

            # ---- banded conv sweep + combine (o_br = psA*g + psB), bo-pairs
            o_sb = opool.tile([128, 2, SP], dt.bfloat16, tag="o")
            for bop in range(13):
                w = 2 if bop < 12 else 1
                ps = ps_sq.tile([128, 2, 4, 128], dt.float32, tag="sq")
                for p in range(w):
                    bo = 2 * bop + p
                    ph = bo % PH
                    if SWEEP_FP8:
                        nc.tensor.matmul(ps[:, p], fT[:, bo:bo + 2, :], Tdr[:, ph],
                                         start=True, stop=False, perf_mode=DR,
                                         skip_group_check=True)
                        nc.tensor.matmul(ps[:, p], fT[:, bo + 2, :], Tsg[:, ph],
                                         start=False, stop=True,
                                         skip_group_check=True)
                    else:
                        for pos in range(3):
                            nc.tensor.matmul(ps[:, p], fT[:, bo + pos, :],
                                             Tbf[:, ph, pos],
                                             start=(pos == 0), stop=(pos == 2))
                # per branch: o = psA*g + psB over the bo-pair (3D APs only;
                # gpsimd cannot read PSUM, so op1 alternates Act/DVE)
                for br in range(2):
                    dst = o_sb[:, br, 2 * bop * 128:(2 * bop + w) * 128]
                    psA = ps[:, 0:w, br, :]
                    psB = ps[:, 0:w, 2 + br, :]
                    if (bop + br) % 2 == 0:
                        nc.scalar.activation(dst, psB, AF.Copy)
                    else:
                        nc.vector.tensor_copy(dst, psB)
                    nc.vector.scalar_tensor_tensor(dst, psA, g[:, 0:1], dst,
                                                   ALU.mult, ALU.add)

            # ---- fuse + y out
            for br in range(2):
                for och in range(2):
                    ysb = ypool.tile([128, HW], dt.bfloat16, tag="y")
                    for sch in range(7):
                        ps = ps_fu.tile([128, SCH], dt.float32, tag="fu")
                        nc.tensor.matmul(ps, wfuseT[:, och, :],
                                         o_sb[:, br, sch * SCH:(sch + 1) * SCH],
                                         start=True, stop=True)
                        dst = ysb[:, sch * SCH:(sch + 1) * SCH]
                        k = (br * 2 + och) * 7 + sch
                        if k % 7 < 4:
                            nc.scalar.activation(dst, ps, AF.Identity,
                                                 bias=bfuseT[:, och:och + 1],
                                                 scale=1.0)
                        else:
                            nc.vector.tensor_scalar_add(dst, ps,
                                                        bfuseT[:, och:och + 1])
                    nc.sync.dma_start(
                        out=y_d[n, br * 256 + och * 128:br * 256 + och * 128 + 128, :],
                        in_=ysb)

    nc.compile()
    return nc


def _get_module():
    if "nc" not in _CACHE:
        _CACHE["nc"] = _build_module()
    return _CACHE["nc"]


# ---------------------------------------------------------------- entry point
def _run(inputs, trace=False, **kwargs):
    from concourse.bass_utils import run_bass_kernel_spmd

    import ml_dtypes

    nc = _get_module()
    consts = _host_consts(inputs)
    x = np.asarray(inputs["x"], np.float32).reshape(N, C, HW)
    if not X_F32:
        x = x.astype(ml_dtypes.bfloat16)
    in_maps = []
    for i in range(NCORES):
        m = dict(consts)
        m["x"] = np.ascontiguousarray(x[i * NPC:(i + 1) * NPC])
        in_maps.append(m)
    return run_bass_kernel_spmd(nc, in_maps, core_ids=list(range(NCORES)),
                                trace=trace, **kwargs)


def kernel(**inputs):
    res = _run(inputs)
    y = np.concatenate([np.asarray(r["y"], np.float32) for r in res.results], axis=0)
    return y.reshape(N, 2 * P2, H, W)


if __name__ == "__main__":
    rng = np.random.default_rng(0)
    demo = {
        "x": rng.standard_normal((N, C, H, W), np.float32),
        "W_conv": 0.05 * rng.standard_normal((CM, C)).astype(np.float32),
        "b_conv": 0.05 * rng.standard_normal(CM).astype(np.float32),
        "wk": 0.05 * rng.standard_normal(25).astype(np.float32),
        "bk": 0.05 * rng.standard_normal(25).astype(np.float32),
        "wck": np.float32(0.03), "bck": np.float32(0.01),
        "wk2": 0.05 * rng.standard_normal(9).astype(np.float32),
        "bk2": 0.05 * rng.standard_normal(9).astype(np.float32),
        "wck2": np.float32(0.02), "bck2": np.float32(-0.01),
        "W_fuse": 0.05 * rng.standard_normal((P2, CM)).astype(np.float32),
        "b_fuse": 0.05 * rng.standard_normal(P2).astype(np.float32),
    }
    out = kernel(**demo)
    print(out.shape, out.dtype)


# revision 19
# speedup vs baseline: 1.0064x; 1.0064x over previous
"""Trainium2 Bass kernel for nn_DIDAModule (dense_cnn).

Math: the per-sample "dynamic" depthwise kernels are affine in the channel
gate g:  kern1 = g*A1 + B1  with  A1 = wk*wck, B1 = bk*wck + bck  (5x5) and
A2 = wk2*wck2, B2 = bk2*wck2 + bck2 (3x3, dilation 2).  A/B are shared across
channels, and g is constant per (sample, channel), so the gate commutes with
the spatial conv:

    o1 = g * conv_A1(f) + conv_B1(f)      o2 = g * conv_A2(f) + conv_B2(f)
    y  = [W_fuse @ o1 + b_fuse ; W_fuse @ o2 + b_fuse]

Only ONE feature map (f) is ever convolved.  Pipeline per sample:
  1. conv1 (1x1): stationary W_conv^T chunks, moving x (f32r, full rate),
     psum -> relu -> channel-major f (bf16).  The gate g comes from DVE
     reductions of the same psums (pre-relu spatial mean).
  2. PE-transpose f into spatial-major fT blocks (128-pixel flat blocks,
     25 blocks of the 56x56 map padded to 3200), staged via PSUM.
  3. Banded-conv sweep: stationary fT blocks, moving host-built band
     matrices T[phase,pos] for the 4 static kernels (A1,A2,B1,B2) -> psum
     [c, 4*128] per output block, accumulated over 3 input-block positions
     (fp8 DoubleRow pairs two positions per matmul when SWEEP_FP8).
  4. Combine on DVE: o_br = psA*g + psB (scalar_tensor_tensor), bf16.
  5. Fuse: stationary W_fuse^T chunks, moving o (channel-major), bias via
     activation, y out in bf16.

No DMA transposes anywhere (the xbar path needs fences + delay chains on
this HW); the only DRAM traffic is x in and y out.

Sharding: data-parallel over batch N across the 8 cores (4 samples each),
weights replicated.
"""

import os
import numpy as np

# ---------------------------------------------------------------- dims
N, C, H, W = 32, 512, 56, 56
CM, K1, K2, P2 = 128, 5, 3, 256
HW = H * W            # 3136
SP = 3200             # padded spatial: 25 blocks of 128
NB = 25
PH = 7                # phase classes (128 mod 56 = 16, period 7)
NCORES = 8
NPC = N // NCORES     # samples per core
SCH = 448             # conv free chunk: 3136 = 7*448
TSCALE = 4096.0       # fp8 band-matrix scale (folded out of W_fuse)

SWEEP_FP8 = bool(int(os.environ.get("CCK_FP8", "0")))  # fp8 f costs 1.9e-2 rel err
# x is cast to bf16 on the host: halves x DMA and keeps conv1 all-bf16
# (mixed f32r x bf16 matmuls are rejected by the compiler, NCC_IBIR034).
X_F32 = bool(int(os.environ.get("CCK_XF32", "0")))

_CACHE = {}


# ---------------------------------------------------------------- host prep
def _build_T(K2d, dil):
    """Banded conv matrices T[phase, pos, k_in, m_out] for flat 128-blocks."""
    kh = K2d.shape[0]
    r = (kh - 1) // 2 * dil
    T = np.zeros((PH, 3, 128, 128), np.float32)
    for p in range(PH):
        bref = 7 + p              # interior reference block of this phase
        for pos, d in enumerate((-1, 0, 1)):
            for m in range(128):
                s_out = bref * 128 + m
                ro, wo = divmod(s_out, W)
                for k in range(128):
                    s_in = (bref + d) * 128 + k
                    ri, wi = divmod(s_in, W)
                    di, dj = ri - ro, wi - wo
                    if (abs(di) <= r and abs(dj) <= r
                            and di % dil == 0 and dj % dil == 0):
                        T[p, pos, k, m] = K2d[di // dil + (kh - 1) // 2,
                                              dj // dil + (kh - 1) // 2]
    return T


def _host_consts(inp):
    import ml_dtypes
    bf16 = ml_dtypes.bfloat16
    f8 = ml_dtypes.float8_e4m3
    W_conv = np.asarray(inp["W_conv"], np.float32)     # [CM, C]
    W_fuse = np.asarray(inp["W_fuse"], np.float32)     # [P2, CM]
    A1 = (np.asarray(inp["wk"]) * float(inp["wck"])).reshape(K1, K1)
    B1 = (np.asarray(inp["bk"]) * float(inp["wck"]) + float(inp["bck"])).reshape(K1, K1)
    A2 = (np.asarray(inp["wk2"]) * float(inp["wck2"])).reshape(K2, K2)
    B2 = (np.asarray(inp["bk2"]) * float(inp["wck2"]) + float(inp["bck2"])).reshape(K2, K2)
    # kid order (A1, A2, B1, B2) so the combine can slice A=[0:2], B=[2:4]
    T4 = np.stack([_build_T(A1.astype(np.float32), 1),
                   _build_T(A2.astype(np.float32), 2),
                   _build_T(B1.astype(np.float32), 1),
                   _build_T(B2.astype(np.float32), 2)])   # [kid, ph, pos, k, m]
    # conv1 lhsT chunks: [c_local(128part), kc(4), cm(128)]
    wconvT_h = np.ascontiguousarray(
        W_conv.T.reshape(4, 128, CM).transpose(1, 0, 2))
    # fuse lhsT chunks: [c(128part), chunk(2), o_local(128)]
    wfuseT_h = np.ascontiguousarray(W_fuse.T.reshape(CM, 2, 128))
    d = {
        "wconvT": wconvT_h.astype(np.float32 if X_F32 else bf16),
        "bconv": np.asarray(inp["b_conv"], np.float32).reshape(CM, 1),
        "bfuseT": np.ascontiguousarray(
            np.asarray(inp["b_fuse"], np.float32).reshape(2, 128).T),  # [128, 2]
        "ident": np.eye(128, dtype=bf16),
    }
    if SWEEP_FP8:
        # DoubleRow pairs (pos0, pos1); pos2 rides a plain fp8 matmul.
        Tdr = np.ascontiguousarray(
            (T4[:, :, 0:2] * TSCALE).transpose(3, 1, 2, 0, 4)).astype(f8)
        Tsg = np.ascontiguousarray(
            (T4[:, :, 2] * TSCALE).transpose(2, 1, 0, 3)).astype(f8)
        d["Tdr"] = Tdr                       # [k, ph, t, kid, m]
        d["Tsg"] = Tsg                       # [k, ph, kid, m]
        d["wfuseT"] = (wfuseT_h / TSCALE).astype(bf16)
    else:
        d["Tbf"] = np.ascontiguousarray(
            T4.transpose(3, 1, 2, 0, 4)).astype(bf16)   # [k, ph, pos, kid, m]
        d["wfuseT"] = wfuseT_h.astype(bf16)
    return d


# ---------------------------------------------------------------- bass module
def _build_module():
    from contextlib import ExitStack
    import concourse.bass as bass  # noqa: F401
    import concourse.mybir as mybir
    import concourse.tile as tile
    from concourse import bacc

    dt = mybir.dt
    AX = mybir.AxisListType
    AF = mybir.ActivationFunctionType
    ALU = mybir.AluOpType
    DR = mybir.MatmulPerfMode.DoubleRow

    nc = bacc.Bacc("TRN2", target_bir_lowering=False, debug=False)

    reps = int(os.environ.get("CCK_REPS", "1"))
    x_dt = dt.float32r if X_F32 else dt.bfloat16
    w_dt = x_dt
    f8_dt = dt.float8e4

    x_d = nc.dram_tensor("x", [NPC, C, HW], x_dt, kind="ExternalInput").ap()
    wconvT_d = nc.dram_tensor("wconvT", [128, 4, CM], w_dt, kind="ExternalInput").ap()
    bconv_d = nc.dram_tensor("bconv", [CM, 1], dt.float32, kind="ExternalInput").ap()
    bfuseT_d = nc.dram_tensor("bfuseT", [128, 2], dt.float32, kind="ExternalInput").ap()
    ident_d = nc.dram_tensor("ident", [128, 128], dt.bfloat16, kind="ExternalInput").ap()
    wfuseT_d = nc.dram_tensor("wfuseT", [CM, 2, 128], dt.bfloat16, kind="ExternalInput").ap()
    if SWEEP_FP8:
        Tdr_d = nc.dram_tensor("Tdr", [128, PH, 2, 4, 128], f8_dt, kind="ExternalInput").ap()
        Tsg_d = nc.dram_tensor("Tsg", [128, PH, 4, 128], f8_dt, kind="ExternalInput").ap()
    else:
        Tbf_d = nc.dram_tensor("Tbf", [128, PH, 3, 4, 128], dt.bfloat16, kind="ExternalInput").ap()
    y_d = nc.dram_tensor("y", [NPC, 2 * P2, HW], dt.bfloat16, kind="ExternalOutput").ap()

    with tile.TileContext(nc) as tc, ExitStack() as ctx:
        consts = ctx.enter_context(tc.tile_pool(name="consts", bufs=1))
        xpool = ctx.enter_context(tc.tile_pool(name="xp", bufs=2))
        fpool = ctx.enter_context(tc.tile_pool(name="fp", bufs=2))
        opool = ctx.enter_context(tc.tile_pool(name="op", bufs=2))
        ypool = ctx.enter_context(tc.tile_pool(name="yp", bufs=3))
        small = ctx.enter_context(tc.tile_pool(name="sm", bufs=2))
        ps_c1 = ctx.enter_context(tc.tile_pool(name="psc1", bufs=2, space="PSUM"))
        ps_sq = ctx.enter_context(tc.tile_pool(name="pssq", bufs=2, space="PSUM"))
        ps_fu = ctx.enter_context(tc.tile_pool(name="psfu", bufs=2, space="PSUM"))

        # ---- constants to SBUF (conv1 consts first; big T matrices last so
        # the sample-0 x load, on the gpsimd queue, isn't the startup critical
        # path and conv1 can begin as soon as wconvT lands)
        wconvT = consts.tile([128, 4, CM], w_dt)
        nc.sync.dma_start(out=wconvT, in_=wconvT_d)
        bconv = consts.tile([CM, 1], dt.float32)
        nc.sync.dma_start(out=bconv, in_=bconv_d)
        bfuseT = consts.tile([128, 2], dt.float32)
        nc.sync.dma_start(out=bfuseT, in_=bfuseT_d)
        ident = consts.tile([128, 128], dt.bfloat16)
        nc.sync.dma_start(out=ident, in_=ident_d)
        wfuseT = consts.tile([CM, 2, 128], dt.bfloat16)
        nc.sync.dma_start(out=wfuseT, in_=wfuseT_d)
        if SWEEP_FP8:
            Tdr = consts.tile([128, PH, 2, 4, 128], f8_dt)
            nc.sync.dma_start(out=Tdr, in_=Tdr_d)
            Tsg = consts.tile([128, PH, 4, 128], f8_dt)
            nc.sync.dma_start(out=Tsg, in_=Tsg_d)
        else:
            Tbf = consts.tile([128, PH, 3, 4, 128], dt.bfloat16)
            nc.sync.dma_start(out=Tbf, in_=Tbf_d)

        fT_dt = f8_dt if SWEEP_FP8 else dt.bfloat16

        for rep in range(reps):
          for n in range(NPC):
            # ---- x in (gpsimd queue: decoupled from y stores on sync; first
            # half-chunks land early so conv1 starts sooner)
            xt = xpool.tile([128, 4, HW], x_dt, tag="x")
            for kc in range(4):
                nc.gpsimd.dma_start(out=xt[:, kc, 0:HW // 2],
                                    in_=x_d[n, kc * 128:(kc + 1) * 128, 0:HW // 2])
            for kc in range(4):
                nc.gpsimd.dma_start(out=xt[:, kc, HW // 2:HW],
                                    in_=x_d[n, kc * 128:(kc + 1) * 128, HW // 2:HW])

            # ---- conv1 (channel-major f) + gate pieces
            f_cm = fpool.tile([128, SP], dt.bfloat16, tag="fcm")
            nc.gpsimd.memset(f_cm[:, HW:SP], 0.0)
            gpart = small.tile([128, 8], dt.float32, tag="gp")
            for sch in range(7):
                ps = ps_c1.tile([128, SCH], dt.float32, tag="c1")
                for kc in range(4):
                    nc.tensor.matmul(ps, wconvT[:, kc, :],
                                     xt[:, kc, sch * SCH:(sch + 1) * SCH],
                                     start=(kc == 0), stop=(kc == 3))
                nc.vector.reduce_sum(gpart[:, sch:sch + 1], ps, axis=AX.X)
                nc.scalar.activation(f_cm[:, sch * SCH:(sch + 1) * SCH], ps,
                                     AF.Relu, bias=bconv[:, 0:1], scale=1.0)
            gsum = small.tile([128, 1], dt.float32, tag="gs")
            nc.vector.reduce_sum(gsum, gpart[:, 0:7], axis=AX.X)
            g = small.tile([128, 1], dt.float32, tag="g")
            nc.scalar.activation(g, gsum, AF.Relu, bias=bconv[:, 0:1],
                                 scale=1.0 / HW)

            # ---- PE transpose into spatial-major fT (pad blocks 0/26 zero)
            fT = fpool.tile([128, NB + 2, 128], fT_dt, tag="fT")
            nc.gpsimd.memset(fT[:, 0, :], 0.0)
            nc.gpsimd.memset(fT[:, NB + 1, :], 0.0)
            for grp in range(4):
                w = 8 if grp < 3 else 1
                pst = ps_sq.tile([128, 2, 4, 128], dt.bfloat16, tag="sq")
                for b in range(w):
                    bo = 8 * grp + b
                    nc.tensor.matmul(pst[:, b // 4, b % 4, :],
                                     f_cm[:, bo * 128:(bo + 1) * 128],
                                     ident, is_transpose=True, skip_group_check=True)
                dst = fT[:, 1 + 8 * grp:1 + 8 * grp + w, :]
                src = pst.rearrange("p a b m -> p (a b) m")[:, 0:w, :]
                if grp % 2 == 0:
                    nc.scalar.activation(dst, src, AF.Copy)
                else:
                    nc.vector.tensor_copy(dst, src)

            # ---- banded conv sweep + combine (o_br = psA*g + psB), bo-pairs
            o_sb = opool.tile([128, 2, SP], dt.bfloat16, tag="o")
            for bop in range(13):
                w = 2 if bop < 12 else 1
                ps = ps_sq.tile([128, 2, 4, 128], dt.float32, tag="sq")
                for p in range(w):
                    bo = 2 * bop + p
                    ph = bo % PH
                    if SWEEP_FP8:
                        nc.tensor.matmul(ps[:, p], fT[:, bo:bo + 2, :], Tdr[:, ph],
                                         start=True, stop=False, perf_mode=DR,
                                         skip_group_check=True)
                        nc.tensor.matmul(ps[:, p], fT[:, bo + 2, :], Tsg[:, ph],
                                         start=False, stop=True,
                                         skip_group_check=True)
                    else:
                        # pos1 first with start=True (writes the full region;
                        # pos0/pos2 band pieces only touch 114 of 128 cols).
                        # Order pos2 last so its stationary fT[bo+2] is shared
                        # back-to-back with the next bo's pos1.
                        nc.tensor.matmul(ps[:, p], fT[:, bo + 1, :],
                                         Tbf[:, ph, 1],
                                         start=True, stop=False)
                        nc.tensor.matmul(ps[:, p, :, 0:114], fT[:, bo, :],
                                         Tbf[:, ph, 0, :, 0:114],
                                         start=False, stop=False,
                                         skip_group_check=True)
                        nc.tensor.matmul(ps[:, p, :, 14:128], fT[:, bo + 2, :],
                                         Tbf[:, ph, 2, :, 14:128],
                                         start=False, stop=(True),
                                         skip_group_check=True)
                # per branch: o = psA*g + psB over the bo-pair (3D APs only;
                # gpsimd cannot read PSUM, so op1 alternates Act/DVE)
                for br in range(2):
                    dst = o_sb[:, br, 2 * bop * 128:(2 * bop + w) * 128]
                    psA = ps[:, 0:w, br, :]
                    psB = ps[:, 0:w, 2 + br, :]
                    if (bop + br) % 2 == 0:
                        nc.scalar.activation(dst, psB, AF.Copy)
                    else:
                        nc.vector.tensor_copy(dst, psB)
                    nc.vector.scalar_tensor_tensor(dst, psA, g[:, 0:1], dst,
                                                   ALU.mult, ALU.add)

            # ---- fuse + y out
            for br in range(2):
                for och in range(2):
                    ysb = ypool.tile([128, HW], dt.bfloat16, tag="y")
                    for sch in range(7):
                        ps = ps_fu.tile([128, SCH], dt.float32, tag="fu")
                        nc.tensor.matmul(ps, wfuseT[:, och, :],
                                         o_sb[:, br, sch * SCH:(sch + 1) * SCH],
                                         start=True, stop=True)
                        dst = ysb[:, sch * SCH:(sch + 1) * SCH]
                        k = (br * 2 + och) * 7 + sch
                        if k % 7 < 4:
                            nc.scalar.activation(dst, ps, AF.Identity,
                                                 bias=bfuseT[:, och:och + 1],
                                                 scale=1.0)
                        else:
                            nc.vector.tensor_scalar_add(dst, ps,
                                                        bfuseT[:, och:och + 1])
                    yeng = nc.sync if (br * 2 + och) % 2 == 0 else nc.scalar
                    yeng.dma_start(
                        out=y_d[n, br * 256 + och * 128:br * 256 + och * 128 + 128, :],
                        in_=ysb)

    nc.compile()
    return nc


def _get_module():
    if "nc" not in _CACHE:
        _CACHE["nc"] = _build_module()
    return _CACHE["nc"]


# ---------------------------------------------------------------- entry point
def _run(inputs, trace=False, **kwargs):
    from concourse.bass_utils import run_bass_kernel_spmd

    import ml_dtypes

    nc = _get_module()
    consts = _host_consts(inputs)
    x = np.asarray(inputs["x"], np.float32).reshape(N, C, HW)
    if not X_F32:
        x = x.astype(ml_dtypes.bfloat16)
    in_maps = []
    for i in range(NCORES):
        m = dict(consts)
        m["x"] = np.ascontiguousarray(x[i * NPC:(i + 1) * NPC])
        in_maps.append(m)
    return run_bass_kernel_spmd(nc, in_maps, core_ids=list(range(NCORES)),
                                trace=trace, **kwargs)


def kernel(**inputs):
    res = _run(inputs)
    y = np.concatenate([np.asarray(r["y"], np.float32) for r in res.results], axis=0)
    return y.reshape(N, 2 * P2, H, W)


if __name__ == "__main__":
    rng = np.random.default_rng(0)
    demo = {
        "x": rng.standard_normal((N, C, H, W), np.float32),
        "W_conv": 0.05 * rng.standard_normal((CM, C)).astype(np.float32),
        "b_conv": 0.05 * rng.standard_normal(CM).astype(np.float32),
        "wk": 0.05 * rng.standard_normal(25).astype(np.float32),
        "bk": 0.05 * rng.standard_normal(25).astype(np.float32),
        "wck": np.float32(0.03), "bck": np.float32(0.01),
        "wk2": 0.05 * rng.standard_normal(9).astype(np.float32),
        "bk2": 0.05 * rng.standard_normal(9).astype(np.float32),
        "wck2": np.float32(0.02), "bck2": np.float32(-0.01),
        "W_fuse": 0.05 * rng.standard_normal((P2, CM)).astype(np.float32),
        "b_fuse": 0.05 * rng.standard_normal(P2).astype(np.float32),
    }
    out = kernel(**demo)
    print(out.shape, out.dtype)
